# revision 37
# baseline (speedup 1.0000x reference)
"""Pendulum2 DAE kernel for Trainium2 (Bass/Tile), data-parallel over 8 cores.

Closed form per sample (coords = [x0 x1 x2 x3 v0 v1 v2 v3], M0=M1=G=10):
  d0 = x0-x2, d1 = x1-x3, w0 = v0-v2, w1 = v1-v3
  s1 = x0^2+x1^2, q = x0*d0+x1*d1, r = d0^2+d1^2
  h  = v0^2+v1^2 - 10*x1, k = w0^2+w1^2
  D  = 2*s1*r - q^2
  mu1 = (2*r*h - q*k)/D, mu2 = (s1*k - q*h)/D
  out = [v0 v1 v2 v3,
         -(x0*mu1+d0*mu2), -10-(x1*mu1+d1*mu2), d0*mu2, -10+d1*mu2]

v9 design notes (from microbenchmark calibration + trace analysis):
 - Creation order IS the dependency order Tile tracks: every instruction is
   created after its producers.
 - Strided / broadcast APs are free on DVE and ACT; ops read the interleaved
   (t e) input view directly (no packing copies).
 - GpSimd shares the DVE SBUF port and mislowers 3-level strided views, so it
   only gets cb-broadcast bias adds / plain packed ops (baseline-proven).
 - All five pair-sums run as ONE fused DVE TT over the square planes.
 - Mid-chain (squares, pair-sums, numerator products, mus) in bf16: plane-pair
   and outer-broadcast operands keep innermost step 1, so DVE hits the 2x_1P
   mode. d01, the D chain (s1r2, tq, D, lnD) and the combine stay f32;
   emulated end-to-end error of this split is 7.6e-3 vs the 2e-2 gate.
 - sqD uses scale=sqrt(2) so its pair-sum is r2=2r; num1n = qk-2rh = -mu1*D
   makes the output stage plain subtracts; 1/D = exp(-ln(D)) (D >= s1*r > 0).
 - Asymmetric tiles [256, 512, 512, 512, 256]: small first tile cuts DMA fill
   latency, small last tile cuts the drain.
"""

import json

import numpy as np

from concourse import bass, bass_utils, mybir
from concourse.tile import TileContext


def _split_multi_waits(mod):
    # walrus encodes at most one sync wait per instruction; hoist extra waits
    # onto wait-only EventSemaphore nops on the same engine (in-order issue
    # preserves semantics).
    ctr = 0
    for fn in mod.get("functions", []):
        for blk in fn.get("blocks", []):
            new = []
            for inst in blk.get("instructions", []):
                si = inst.get("sync_info") or {}
                ow = si.get("on_wait") or []
                if len(ow) > 1:
                    for w in ow[:-1]:
                        ctr += 1
                        new.append(
                            {
                                "debug": inst.get("debug", 0),
                                "engine": inst["engine"],
                                "ins": [],
                                "name": f"syncsplit-{ctr}-{inst['name']}",
                                "opcode": "EventSemaphore",
                                "outs": [],
                                "sync_info": {"on_wait": [w]},
                            }
                        )
                    si = dict(si)
                    si["on_wait"] = [ow[-1]]
                    inst = dict(inst)
                    inst["sync_info"] = si
                new.append(inst)
            blk["instructions"] = new
    return mod


_ORIG_TO_JSON_BYTES = bass.Bass.to_json_bytes


def _patched_to_json_bytes(self):
    return json.dumps(_split_multi_waits(json.loads(_ORIG_TO_JSON_BYTES(self)))).encode()


bass.Bass.to_json_bytes = _patched_to_json_bytes

BS = 2_097_152
NCORES = 8
PER = BS // NCORES          # samples per core
P = 128                     # SBUF partitions
TMAX = 512
TILES = [128, 384, 512, 512, 384, 128]   # samples per partition-row per tile
assert sum(TILES) * P == PER

f32 = mybir.dt.float32
bf16 = mybir.dt.bfloat16
ALU = mybir.AluOpType
ACTF = mybir.ActivationFunctionType
SQRT2 = float(np.sqrt(2.0))

# bf16 scratch plane map (TMAX elems each; first tt used):
#  0-1  w01 pk
#  2-11 square pairs [sqD(2-3), m01(4-5), sqX(6-7), sqV(8-9), sqW(10-11)]
#  fused pair-sum -> 12-16 = [r2, q, s1, h1->h, k]
#  17-18 [r2h|qh]   19-20 [qk|s1k]   21-22 [num1n|num2]
#  23 invD   24-25 [mu1n|mu2]
NB = 23
# f32 scratch plane map: 0-1 d01 pk, 2 tq, 3 s1r2/D, 4 lnD, 5-6 U01 pk
NF = 7


def _build():
    nc = bass.Bass()
    coords = nc.dram_tensor("coords", [PER, 8], f32, kind="ExternalInput")
    out = nc.dram_tensor("out", [PER, 8], f32, kind="ExternalOutput")

    with TileContext(nc) as tc:
        with tc.tile_pool(
            name="inp", bufs=4
        ) as inp, tc.tile_pool(name="outp", bufs=2) as outp, tc.tile_pool(
            name="sc", bufs=3
        ) as scp:
            off = 0
            for tidx, tt in enumerate(TILES):
                dram_in = coords[off : off + P * tt].rearrange("(p t) e -> p (t e)", p=P)
                dram_out = out[off : off + P * tt].rearrange("(p t) e -> p (t e)", p=P)
                off += P * tt

                in_full = inp.tile([P, TMAX * 8], f32)
                out_full = outp.tile([P, TMAX * 8], f32)
                sb = scp.tile([P, NB * TMAX], bf16)
                sf = scp.tile([P, NF * TMAX], f32)
                in_t = in_full[:, : tt * 8]
                out_t = out_full[:, : tt * 8]

                nc.sync.dma_start(out=in_t, in_=dram_in)

                iv = in_t.rearrange("p (t e) -> p t e", e=8)
                ovt = out_t.rearrange("p (t e) -> p t e", e=8)

                def bpk(a):
                    return sb[:, a * TMAX : a * TMAX + 2 * tt].rearrange(
                        "p (t e) -> p t e", e=2
                    )

                def bpl(j):
                    return sb[:, j * TMAX : j * TMAX + tt]

                def bpls(a, b):
                    return sb[:, a * TMAX : b * TMAX].rearrange(
                        "p (c t) -> p c t", t=TMAX
                    )[:, :, :tt]

                def bco(j):
                    return (
                        bpl(j).rearrange("p (o t) -> p o t", o=1).broadcast_to((P, 2, tt))
                    )

                def bc2(j):
                    return (
                        bpl(j).rearrange("p (t o) -> p t o", o=1).broadcast_to((P, tt, 2))
                    )

                def fpk(a):
                    return sf[:, a * TMAX : a * TMAX + 2 * tt].rearrange(
                        "p (t e) -> p t e", e=2
                    )

                def fpl(j):
                    return sf[:, j * TMAX : j * TMAX + tt]

                V, S, G = nc.vector, nc.scalar, nc.gpsimd

                # diffs + squares (squares land in bf16)
                V.tensor_sub(out=fpk(0), in0=iv[:, :, 0:2], in1=iv[:, :, 2:4])   # d01 (f32)
                V.tensor_sub(out=bpk(0), in0=iv[:, :, 4:6], in1=iv[:, :, 6:8])   # w01
                S.activation(bpk(2), fpk(0), ACTF.Square, scale=SQRT2)           # sqD = 2d^2
                S.activation(bpk(6), iv[:, :, 0:2], ACTF.Square)                 # sqX
                S.activation(bpk(8), iv[:, :, 4:6], ACTF.Square)                 # sqV
                S.activation(bpk(10), bpk(0), ACTF.Square)                       # sqW
                S.copy(ovt[:, :, 0:4], iv[:, :, 4:8])                            # v passthrough
                V.tensor_tensor(out=bpk(4), in0=iv[:, :, 0:2], in1=fpk(0), op=ALU.mult)  # m01

                # all five pair-sums in one TT: bf16 planes 2-11 even vs odd
                sq5 = (
                    sb[:, 2 * TMAX : 12 * TMAX]
                    .rearrange("p (c r) -> p c r", c=5)[:, :, : 2 * tt]
                    .rearrange("p c (t e) -> p c t e", e=2)
                )
                ps = sb[:, 12 * TMAX : 17 * TMAX].rearrange("p (c t) -> p c t", c=5)[
                    :, :, :tt
                ]
                V.tensor_add(out=ps, in0=sq5[:, :, :, 0], in1=sq5[:, :, :, 1])   # [r2,q,s1,h1,k]

                # h = h1 - 10*x1: t10 on ACT, then a 2x bf16 add on V
                S.activation(bpl(3), iv[:, :, 1], ACTF.Copy, scale=-10.0)        # t10
                V.tensor_add(out=bpl(15), in0=bpl(15), in1=bpl(3))               # h
                V.tensor_tensor(out=bpls(17, 19), in0=bpls(12, 14), in1=bco(15), op=ALU.mult)  # [r2h|qh]
                V.tensor_tensor(out=bpls(19, 21), in0=bpls(13, 15), in1=bco(16), op=ALU.mult)  # [qk|s1k]
                V.tensor_sub(out=bpls(21, 23), in0=bpls(19, 21), in1=bpls(17, 19))  # [num1n|num2]

                # D chain (f32 arithmetic on bf16 inputs)
                S.activation(fpl(2), bpl(13), ACTF.Square)                       # tq = q^2
                V.tensor_tensor(out=fpl(3), in0=bpl(14), in1=bpl(12), op=ALU.mult)  # s1*r2
                V.tensor_sub(out=fpl(3), in0=fpl(3), in1=fpl(2))                 # D
                S.activation(fpl(4), fpl(3), ACTF.Ln)                            # ln D
                S.activation(bpl(2), fpl(4), ACTF.Exp, scale=-1.0)               # invD (bf16)
                V.tensor_tensor(out=bpls(0, 2), in0=bpls(21, 23), in1=bco(2), op=ALU.mult)  # [mu1n|mu2]

                # combine: write an / U01-an straight to the output tile, then
                # apply the -10 biases to the a1/a3 columns as one ACT op
                V.tensor_tensor(out=ovt[:, :, 6:8], in0=fpk(0), in1=bc2(1), op=ALU.mult)  # [a2, d1mu2]
                V.tensor_tensor(out=fpk(5), in0=iv[:, :, 0:2], in1=bc2(0), op=ALU.mult)  # U01
                V.tensor_sub(out=ovt[:, :, 4:6], in0=fpk(5), in1=ovt[:, :, 6:8])   # [a0, a1+10]
                ov57 = ovt[:, :, 5:8:2]
                S.activation(ov57, ov57, ACTF.Copy, bias=-10.0)                   # a1/a3 -= 10

                nc.sync.dma_start(out=dram_out, in_=out_t)
    return nc


_NC = None


def _run(coords, trace=False, **kw):
    global _NC
    if _NC is None:
        _NC = _build()
    coords = np.ascontiguousarray(coords, dtype=np.float32)
    in_maps = [
        {"coords": coords[c * PER : (c + 1) * PER]} for c in range(NCORES)
    ]
    res = bass_utils.run_bass_kernel_spmd(
        _NC, in_maps, core_ids=list(range(NCORES)), trace=trace, **kw
    )
    out = np.concatenate([res.results[c]["out"] for c in range(NCORES)], axis=0)
    return out, res


def kernel(t, coords):
    return _run(coords)[0]


# revision 38
# speedup vs baseline: 1.0298x; 1.0298x over previous
"""Pendulum2 DAE kernel for Trainium2 (Bass/Tile), data-parallel over 8 cores.

Closed form per sample (coords = [x0 x1 x2 x3 v0 v1 v2 v3], M0=M1=G=10):
  d0 = x0-x2, d1 = x1-x3, w0 = v0-v2, w1 = v1-v3
  s1 = x0^2+x1^2, q = x0*d0+x1*d1, r = d0^2+d1^2
  h  = v0^2+v1^2 - 10*x1, k = w0^2+w1^2
  D  = 2*s1*r - q^2
  mu1 = (2*r*h - q*k)/D, mu2 = (s1*k - q*h)/D
  out = [v0 v1 v2 v3,
         -(x0*mu1+d0*mu2), -10-(x1*mu1+d1*mu2), d0*mu2, -10+d1*mu2]

v9 design notes (from microbenchmark calibration + trace analysis):
 - Creation order IS the dependency order Tile tracks: every instruction is
   created after its producers.
 - Strided / broadcast APs are free on DVE and ACT; ops read the interleaved
   (t e) input view directly (no packing copies).
 - GpSimd shares the DVE SBUF port and mislowers 3-level strided views, so it
   only gets cb-broadcast bias adds / plain packed ops (baseline-proven).
 - All five pair-sums run as ONE fused DVE TT over the square planes.
 - Mid-chain (squares, pair-sums, numerator products, mus) in bf16: plane-pair
   and outer-broadcast operands keep innermost step 1, so DVE hits the 2x_1P
   mode. d01, the D chain (s1r2, tq, D, lnD) and the combine stay f32;
   emulated end-to-end error of this split is 7.6e-3 vs the 2e-2 gate.
 - sqD uses scale=sqrt(2) so its pair-sum is r2=2r; num1n = qk-2rh = -mu1*D
   makes the output stage plain subtracts; 1/D = exp(-ln(D)) (D >= s1*r > 0).
 - Asymmetric tiles [256, 512, 512, 512, 256]: small first tile cuts DMA fill
   latency, small last tile cuts the drain.
"""

import json

import numpy as np

from concourse import bass, bass_utils, mybir
from concourse.tile import TileContext


def _split_multi_waits(mod):
    # walrus encodes at most one sync wait per instruction; hoist extra waits
    # onto wait-only EventSemaphore nops on the same engine (in-order issue
    # preserves semantics).
    ctr = 0
    for fn in mod.get("functions", []):
        for blk in fn.get("blocks", []):
            new = []
            for inst in blk.get("instructions", []):
                si = inst.get("sync_info") or {}
                ow = si.get("on_wait") or []
                if len(ow) > 1:
                    for w in ow[:-1]:
                        ctr += 1
                        new.append(
                            {
                                "debug": inst.get("debug", 0),
                                "engine": inst["engine"],
                                "ins": [],
                                "name": f"syncsplit-{ctr}-{inst['name']}",
                                "opcode": "EventSemaphore",
                                "outs": [],
                                "sync_info": {"on_wait": [w]},
                            }
                        )
                    si = dict(si)
                    si["on_wait"] = [ow[-1]]
                    inst = dict(inst)
                    inst["sync_info"] = si
                new.append(inst)
            blk["instructions"] = new
    return mod


_ORIG_TO_JSON_BYTES = bass.Bass.to_json_bytes


def _patched_to_json_bytes(self):
    return json.dumps(_split_multi_waits(json.loads(_ORIG_TO_JSON_BYTES(self)))).encode()


bass.Bass.to_json_bytes = _patched_to_json_bytes

BS = 2_097_152
NCORES = 8
PER = BS // NCORES          # samples per core
P = 128                     # SBUF partitions
TMAX = 512
TILES = [128, 384, 512, 512, 448, 64]   # samples per partition-row per tile
assert sum(TILES) * P == PER

f32 = mybir.dt.float32
bf16 = mybir.dt.bfloat16
ALU = mybir.AluOpType
ACTF = mybir.ActivationFunctionType
SQRT2 = float(np.sqrt(2.0))

# bf16 scratch plane map (TMAX elems each; first tt used):
#  0-1  w01 pk
#  2-11 square pairs [sqD(2-3), m01(4-5), sqX(6-7), sqV(8-9), sqW(10-11)]
#  fused pair-sum -> 12-16 = [r2, q, s1, h1->h, k]
#  17-18 [r2h|qh]   19-20 [qk|s1k]   21-22 [num1n|num2]
#  23 invD   24-25 [mu1n|mu2]
NB = 27
# f32 scratch plane map: 0-1 d01 pk, 2 tq, 3 s1r2/D, 4 lnD, 5-6 U01 pk
NF = 7


def _build():
    nc = bass.Bass()
    coords = nc.dram_tensor("coords", [PER, 8], f32, kind="ExternalInput")
    out = nc.dram_tensor("out", [PER, 8], f32, kind="ExternalOutput")

    with TileContext(nc) as tc:
        with tc.tile_pool(
            name="inp", bufs=3
        ) as inp, tc.tile_pool(name="outp", bufs=2) as outp, tc.tile_pool(
            name="sc", bufs=3
        ) as scp:
            off = 0
            for tidx, tt in enumerate(TILES):
                dram_in = coords[off : off + P * tt].rearrange("(p t) e -> p (t e)", p=P)
                dram_out = out[off : off + P * tt].rearrange("(p t) e -> p (t e)", p=P)
                off += P * tt

                in_full = inp.tile([P, TMAX * 8], f32)
                out_full = outp.tile([P, TMAX * 8], f32)
                sb = scp.tile([P, NB * TMAX], bf16)
                sf = scp.tile([P, NF * TMAX], f32)
                in_t = in_full[:, : tt * 8]
                out_t = out_full[:, : tt * 8]

                nc.sync.dma_start(out=in_t, in_=dram_in)

                iv = in_t.rearrange("p (t e) -> p t e", e=8)
                ovt = out_t.rearrange("p (t e) -> p t e", e=8)

                def bpk(a):
                    return sb[:, a * TMAX : a * TMAX + 2 * tt].rearrange(
                        "p (t e) -> p t e", e=2
                    )

                def bpl(j):
                    return sb[:, j * TMAX : j * TMAX + tt]

                def bpls(a, b):
                    return sb[:, a * TMAX : b * TMAX].rearrange(
                        "p (c t) -> p c t", t=TMAX
                    )[:, :, :tt]

                def bco(j):
                    return (
                        bpl(j).rearrange("p (o t) -> p o t", o=1).broadcast_to((P, 2, tt))
                    )

                def bc2(j):
                    return (
                        bpl(j).rearrange("p (t o) -> p t o", o=1).broadcast_to((P, tt, 2))
                    )

                def fpk(a):
                    return sf[:, a * TMAX : a * TMAX + 2 * tt].rearrange(
                        "p (t e) -> p t e", e=2
                    )

                def fpl(j):
                    return sf[:, j * TMAX : j * TMAX + tt]

                V, S, G = nc.vector, nc.scalar, nc.gpsimd

                # diffs + squares (squares land in bf16)
                V.tensor_sub(out=fpk(0), in0=iv[:, :, 0:2], in1=iv[:, :, 2:4])   # d01 (f32)
                V.tensor_sub(out=bpk(0), in0=iv[:, :, 4:6], in1=iv[:, :, 6:8])   # w01
                S.activation(bpk(2), fpk(0), ACTF.Square, scale=SQRT2)           # sqD = 2d^2
                S.activation(bpk(6), iv[:, :, 0:2], ACTF.Square)                 # sqX
                S.activation(bpk(8), iv[:, :, 4:6], ACTF.Square)                 # sqV
                S.activation(bpk(10), bpk(0), ACTF.Square)                       # sqW
                S.copy(ovt[:, :, 0:4], iv[:, :, 4:8])                            # v passthrough
                V.tensor_tensor(out=bpk(4), in0=iv[:, :, 0:2], in1=fpk(0), op=ALU.mult)  # m01

                # all five pair-sums in one TT: bf16 planes 2-11 even vs odd
                sq5 = (
                    sb[:, 2 * TMAX : 12 * TMAX]
                    .rearrange("p (c r) -> p c r", c=5)[:, :, : 2 * tt]
                    .rearrange("p c (t e) -> p c t e", e=2)
                )
                ps = sb[:, 12 * TMAX : 17 * TMAX].rearrange("p (c t) -> p c t", c=5)[
                    :, :, :tt
                ]
                V.tensor_add(out=ps, in0=sq5[:, :, :, 0], in1=sq5[:, :, :, 1])   # [r2,q,s1,h1,k]

                # h = h1 - 10*x1: t10 on ACT, then a 2x bf16 add on V
                S.activation(bpl(26), iv[:, :, 1], ACTF.Copy, scale=-10.0)       # t10
                V.tensor_add(out=bpl(15), in0=bpl(15), in1=bpl(26))              # h
                V.tensor_tensor(out=bpls(17, 19), in0=bpls(12, 14), in1=bco(15), op=ALU.mult)  # [r2h|qh]
                V.tensor_tensor(out=bpls(19, 21), in0=bpls(13, 15), in1=bco(16), op=ALU.mult)  # [qk|s1k]
                V.tensor_sub(out=bpls(21, 23), in0=bpls(19, 21), in1=bpls(17, 19))  # [num1n|num2]

                # D chain (f32 arithmetic on bf16 inputs)
                S.activation(fpl(2), bpl(13), ACTF.Square)                       # tq = q^2
                V.tensor_tensor(out=fpl(3), in0=bpl(14), in1=bpl(12), op=ALU.mult)  # s1*r2
                V.tensor_sub(out=fpl(3), in0=fpl(3), in1=fpl(2))                 # D
                S.activation(fpl(4), fpl(3), ACTF.Ln)                            # ln D
                S.activation(bpl(23), fpl(4), ACTF.Exp, scale=-1.0)              # invD (bf16)
                V.tensor_tensor(out=bpls(24, 26), in0=bpls(21, 23), in1=bco(23), op=ALU.mult)  # [mu1n|mu2]

                # combine: write an / U01-an straight to the output tile, then
                # apply the -10 biases to the a1/a3 columns as one ACT op
                V.tensor_tensor(out=ovt[:, :, 6:8], in0=fpk(0), in1=bc2(25), op=ALU.mult)  # [a2, d1mu2]
                V.tensor_tensor(out=fpk(5), in0=iv[:, :, 0:2], in1=bc2(24), op=ALU.mult)  # U01
                V.tensor_sub(out=ovt[:, :, 4:6], in0=fpk(5), in1=ovt[:, :, 6:8])   # [a0, a1+10]
                ov57 = ovt[:, :, 5:8:2]
                S.activation(ov57, ov57, ACTF.Copy, bias=-10.0)                   # a1/a3 -= 10

                nc.sync.dma_start(out=dram_out, in_=out_t)
    return nc


_NC = None


def _run(coords, trace=False, **kw):
    global _NC
    if _NC is None:
        _NC = _build()
    coords = np.ascontiguousarray(coords, dtype=np.float32)
    in_maps = [
        {"coords": coords[c * PER : (c + 1) * PER]} for c in range(NCORES)
    ]
    res = bass_utils.run_bass_kernel_spmd(
        _NC, in_maps, core_ids=list(range(NCORES)), trace=trace, **kw
    )
    out = np.concatenate([res.results[c]["out"] for c in range(NCORES)], axis=0)
    return out, res


def kernel(t, coords):
    return _run(coords)[0]
